# revision 50
# baseline (speedup 1.0000x reference)
"""Trainium2 Bass kernel for sparse_attention problem nn_CAMD_73229192397362.

Reference computation (T=8192, D=32, 4 modalities):
  Q = MLP(m1, Wq, bq); for each modality m: K = MLP(m, Wk, bk), V = m,
  Z_m[i] = sum_{j: t2[j] <= t1[i]} (Q_i . K_j) V_j   (causal linear attention)
  out = (sum_m Z_m)[..., :2]                          -> (1, T, 2)

Only V[:, :2] affects the output, so per modality we need
  Z_m[i, v] = Q_i . H_m(r(i)),  H_m(p) = sum_{j<p} K_j (x) V_j[:2]
with r(i) = #{j : t2[j] <= t1[i]} (both timestamp sequences sorted).

Device algorithm per core (8 cores = 4 modalities x 2 interleaved query
half-sets so band metadata is uniform across cores):
  - MLPs run on the PE in transposed layout (D on partitions) with the
    T-chunks stacked 4-up on the 128 partitions; the three layer weights
    are expanded host-side to 128x128 BLOCK-DIAGONAL matrices so each
    512-column group is ONE matmul instruction (not 4 tile_position ones).
  - K in natural layout (knat) is produced by 16 full 128x128 PE
    transposes of the stacked kt_s (each yields 4 key-chunks at once).
  - Chunked causal attention: for each query tile I (128 queries) the key
    range splits into a full prefix [0, w_I) handled through a running
    (32,2) state H accumulated in PSUM, plus a banded window
    [w_I, w_I + W) where S^T = K_band Q_I^T is computed on PE, masked with
    one fused scalar_tensor_tensor (is_ge + mult) on DVE (the H state
    updates run as per-column bias-adds on ACT to stay off the DVE
    queue), and contracted
    against V2 with output-transposed matmuls zot[128q, 2] (tiny moving
    operand => near-free on PE).  w_I / W are computed host-side from the
    timestamps and are identical for all cores by construction.
  - Single merged instruction stream: a PE warm-up chain covers the HAM
    p-state ramp during the input DMAs; the Q MLP runs first (smaller
    input); the K final layer (+1 group lookahead) and deferred
    transposes are emitted lazily per 512-column group, and attention tiles enter a depth-3 software
    pipeline (S^T/masks run ahead of the zot accumulations) as soon as
    the key chunks their band needs exist.  Input DMAs are spread over
    the SP/ACT/GpSimd queues; t1 is broadcast across partitions on-chip
    by the Pool engine instead of loading a 128x-duplicated tensor.
"""

import os
import numpy as np

import concourse.bass as bass
from concourse.bacc import Bacc
import concourse.mybir as mybir
from concourse.tile import TileContext
from concourse.bass_utils import run_bass_kernel_spmd

T = 8192
D = 32
TQ = 4096          # queries per core
NT = TQ // 128     # query tiles per core (32)
NCH = T // 128     # key chunks (64)
F32 = mybir.dt.float32
AF = mybir.ActivationFunctionType
OP = mybir.AluOpType


def _stack4(xT):
    """(32, Ttot) -> (128, Ttot//4): 512-col chunk g goes to partition
    block g%4, columns (g//4)*512."""
    d, Ttot = xT.shape
    ng = Ttot // 512
    out = np.zeros((128, Ttot // 4), dtype=xT.dtype)
    for g in range(ng):
        k = g % 4
        out[32 * k:32 * k + 32, (g // 4) * 512:(g // 4) * 512 + 512] = \
            xT[:, g * 512:(g + 1) * 512]
    return out


def _knat_col(c):
    """Column offset of key-chunk c inside knat (packed-transpose layout)."""
    u, r = c // 16, c % 16
    J = 4 * u + (r % 4)
    k = r // 4
    return 128 * J + 32 * k


def _band_meta(t1_all, t2_all):
    """Uniform band metadata across all (core, tile) combos.

    t1_all: list over cores of (TQ,) query ts; t2_all: list of (T,) key ts.
    Returns (w, W, nb): w[I] 128-aligned band starts (len NT), band width W,
    nb[I] chunks actually needed per tile."""
    w_raw = np.full(NT, T, dtype=np.int64)
    for t1 in t1_all:
        for t2 in t2_all:
            r_min = np.searchsorted(t2, t1[::128], side="right")  # (NT,)
            w_raw = np.minimum(w_raw, (r_min // 128) * 128)
    W = 0
    for t1 in t1_all:
        for t2 in t2_all:
            r_max = np.searchsorted(t2, t1[127::128], side="right")
            W = max(W, int(np.max(r_max - w_raw)))
    W = max(128, ((W + 127) // 128) * 128)
    w = np.minimum(w_raw, T - W)
    assert np.all(np.diff(w) >= 0)
    for t1 in t1_all:
        for t2 in t2_all:
            r_min = np.searchsorted(t2, t1[::128], side="right")
            r_max = np.searchsorted(t2, t1[127::128], side="right")
            assert np.all(w <= r_min) and np.all(r_max <= w + W)
    nb = np.zeros(NT, dtype=np.int64)
    for t1 in t1_all:
        for t2 in t2_all:
            r_max = np.searchsorted(t2, t1[127::128], side="right")
            nb = np.maximum(nb, (r_max - w + 127) // 128)
    nb = np.maximum(nb, 1)
    return [int(x) for x in w], int(W), [int(x) for x in nb]


def _build(w, W, nb):
    """Build the Bass module (same program for all 8 cores)."""
    nc = Bacc("TRN2")

    xk = nc.dram_tensor("xk", [128, T // 4], F32, kind="ExternalInput")
    xq = nc.dram_tensor("xq", [128, TQ // 4], F32, kind="ExternalInput")
    wk = nc.dram_tensor("wk", [128, 384], F32, kind="ExternalInput")
    wq = nc.dram_tensor("wq", [128, 384], F32, kind="ExternalInput")
    bk = nc.dram_tensor("bk", [128, 3], F32, kind="ExternalInput")
    bq = nc.dram_tensor("bq", [128, 3], F32, kind="ExternalInput")
    idm = nc.dram_tensor("idm", [128, 128], F32, kind="ExternalInput")
    t1r = nc.dram_tensor("t1r", [1, TQ], F32, kind="ExternalInput")
    t2p = nc.dram_tensor("t2p", [128, NCH], F32, kind="ExternalInput")
    v2n = nc.dram_tensor("v2n", [128, 2 * NCH], F32, kind="ExternalInput")
    # contiguous per-partition layout; host unscrambles to (TQ, 2)
    out = nc.dram_tensor("out", [128, 2 * NT], F32, kind="ExternalOutput")

    NB = W // 128
    wc = [x // 128 for x in w]           # band start chunk per tile
    nbl = nb
    # GPSIMD cannot access PSUM on TRN2, so Pool-offloaded masks first get
    # an Activation-engine staging copy of S^T into SBUF.
    mask_pool_every = int(os.environ.get("KMASK_POOL_EVERY", "0"))

    with TileContext(nc) as tc:
        with tc.tile_pool(name="cst", bufs=1) as cst, \
             tc.tile_pool(name="big", bufs=1) as big, \
             tc.tile_pool(name="hps", bufs=1, space="PSUM") as hps:

            # --- persistent SBUF tensors ---
            wk_s = cst.tile([128, 384], F32)
            wq_s = cst.tile([128, 384], F32)
            bk_s = cst.tile([128, 3], F32)
            bq_s = cst.tile([128, 3], F32)
            idm_s = cst.tile([128, 128], F32)
            t1r_s = cst.tile([1, TQ], F32)
            t1b_s = big.tile([128, TQ], F32, tag="t1b")
            t2p_s = cst.tile([128, NCH], F32)
            v2n_s = cst.tile([128, 2 * NCH], F32)
            xk_s = big.tile([128, T // 4], F32, tag="xk")
            xq_s = big.tile([128, TQ // 4], F32, tag="xq")
            kt_s = big.tile([128, T // 4], F32, tag="kt")    # K^T stacked
            ktf = big.tile([32, T], F32, tag="ktf")          # K^T flat
            knat = big.tile([128, NCH * 32], F32, tag="knat")  # K natural
            qtf = big.tile([32, TQ], F32, tag="qtf")         # Q^T flat
            zsb = cst.tile([128, 2 * NT], F32)

            # Spread input DMAs across the three DMA-capable queues so they
            # issue and transfer concurrently.  Critical path: xq/wq (Q-MLP
            # start) on ACT, xk/wk (K-MLP) on SP, bulk attention inputs via
            # GpSimd's software DGE (slow generation, async transfer).
            for G in range(2):
                nc.scalar.dma_start(xq_s[:, 512 * G:512 * G + 512],
                                    xq[:, 512 * G:512 * G + 512])
            for dst, src in ((wq_s, wq), (bq_s, bq), (wk_s, wk), (bk_s, bk)):
                nc.sync.dma_start(dst[:], src[:])
            # xk in 4 pieces so K-MLP layer 1 starts before the full load
            for G in range(4):
                nc.sync.dma_start(xk_s[:, 512 * G:512 * G + 512],
                                  xk[:, 512 * G:512 * G + 512])
            for dst, src in ((idm_s, idm), (t2p_s, t2p), (v2n_s, v2n),
                             (t1r_s, t1r)):
                nc.gpsimd.dma_start(dst[:], src[:])
            # broadcast t1 across partitions on the (otherwise idle) Pool
            # engine instead of DMAing a 128x-duplicated 2MB tensor
            for G in range(4):
                nc.gpsimd.partition_broadcast(
                    t1b_s[:, 1024 * G:1024 * G + 1024],
                    t1r_s[:, 1024 * G:1024 * G + 1024])

            # ---------------- merged MLP + attention stream ----------------
            # The K-side MLP runs DEPTH-FIRST per 512-column group (each
            # group yields 16 contiguous key chunks of ktf and knat), and
            # attention tiles are admitted into the software pipeline as
            # soon as the key chunks their band needs are ready.  This
            # overlaps the (PE-bound) MLP tail with the (DVE-bound) mask
            # stream of early attention tiles.
            with tc.tile_pool(name="mlp", bufs=2, space="PSUM") as mlp, \
                 tc.tile_pool(name="hbuf", bufs=2) as hbuf, \
                 tc.tile_pool(name="stp", bufs=3, space="PSUM") as stp, \
                 tc.tile_pool(name="zop", bufs=2, space="PSUM") as zop, \
                 tc.tile_pool(name="smp", bufs=6) as smp, \
                 tc.tile_pool(name="stg", bufs=6) as stgp, \
                 tc.tile_pool(name="hsb", bufs=4) as hsbp:

                # PE warm-up chain: data-independent matmuls during the
                # input-DMA wait so the tensor engine's p-state/HAM ramp
                # (~3.4us of throttled clock) is paid before real work.
                nwarm = int(os.environ.get("KWARM", "14"))
                if nwarm:
                    wz = hbuf.tile([128, 64], F32, tag="warm_src", bufs=1)
                    nc.vector.memset(wz[:], 0)
                    wp = mlp.tile([64, 64], F32, tag="mlp")
                    for _ in range(nwarm):
                        nc.tensor.matmul(wp[:], wz[:], wz[:],
                                         start=True, stop=True)

                def hidden_layers(x_s, w_s, b_s, ngrp, use_act):
                    """L1+L2 breadth-first across groups; returns list of
                    per-group h2 tiles.  use_act: relu on the Activation
                    engine (Q side) vs DVE (K side, keeps the ACT queue
                    clear)."""
                    hs = [x_s[:, 512 * G:512 * G + 512] for G in range(ngrp)]
                    for l in range(2):
                        nxt = []
                        for G in range(ngrp):
                            pt = mlp.tile([128, 512], F32, tag="mlp")
                            nc.tensor.matmul(
                                pt[:], w_s[:, 128 * l:128 * l + 128], hs[G],
                                start=True, stop=True)
                            h = hbuf.tile([128, 512], F32, tag="h", bufs=6,
                                          name=f"h{l}_{G}")
                            if use_act:
                                nc.scalar.activation(h[:], pt[:], AF.Relu,
                                                     bias=b_s[:, l:l + 1])
                            else:
                                nc.vector.tensor_scalar(
                                    h[:], pt[:], b_s[:, l:l + 1], 0.0,
                                    OP.add, OP.max)
                            nxt.append(h[:])
                        hs = nxt
                    return hs

                # Q side first: xq is 4x smaller, so the PE starts (and
                # ramps) sooner while xk still streams in.  Stacked group u
                # covers queries [2048u, 2048u+2048); each 32-row piece is
                # contiguous in qtf.
                qts = hbuf.tile([128, TQ // 4], F32, tag="qts")
                h2q = hidden_layers(xq_s, wq_s, bq_s, 2, True)
                for u in range(2):
                    pt = mlp.tile([128, 512], F32, tag="mlp")
                    nc.tensor.matmul(pt[:], wq_s[:, 256:384], h2q[u],
                                     start=True, stop=True)
                    nc.scalar.activation(
                        qts[:, 512 * u:512 * u + 512], pt[:],
                        AF.Identity, bias=bq_s[:, 2:3])
                    for k in range(4):
                        q0 = (4 * u + k) * 512
                        nc.sync.dma_start(
                            qtf[0:32, q0:q0 + 512],
                            qts[32 * k:32 * k + 32, 512 * u:512 * u + 512])

                h2k = [None] * 4

                def emit_k_l12():
                    h2k[:] = hidden_layers(xk_s, wk_s, bk_s, 4, False)

                def emit_k_final(P):
                    """K final layer + unstack for key chunks [16P,16P+16)."""
                    pt = mlp.tile([128, 512], F32, tag="mlp")
                    nc.tensor.matmul(pt[:], wk_s[:, 256:384], h2k[P],
                                     start=True, stop=True)
                    nc.scalar.activation(
                        kt_s[:, 512 * P:512 * P + 512], pt[:],
                        AF.Identity, bias=bk_s[:, 2:3])
                    for k in range(4):
                        k0 = (4 * P + k) * 512
                        nc.sync.dma_start(
                            ktf[0:32, k0:k0 + 512],
                            kt_s[32 * k:32 * k + 32, 512 * P:512 * P + 512])

                def emit_k_transpose(P):
                    """Natural-layout transpose (deferred so other PE work
                    hides the ACT kt_s writeback latency)."""
                    pt = mlp.tile([128, 512], F32, tag="mlp")
                    for j in range(4):
                        J = 4 * P + j
                        nc.tensor.transpose(
                            pt[:, 128 * j:128 * j + 128],
                            kt_s[:, 128 * J:128 * J + 128], idm_s[:])
                    nc.scalar.activation(
                        knat[:, 512 * P:512 * P + 512], pt[:], AF.Copy)

                hsb = hsbp.tile([32, 2], F32, tag="hsb")
                nc.vector.memset(hsb[:], 0)
                state = {"delta_done": 0, "mask_i": 0}
                hsb_at = {}
                smt_at = {}
                st4_at = {}

                def emit_front_st(I):
                    """H-delta and S^T for tile I (PE work)."""
                    nonlocal hsb
                    if wc[I] > state["delta_done"]:
                        dps = hps.tile([32, 2], F32, tag="dh")
                        for c in range(state["delta_done"], wc[I]):
                            col = _knat_col(c)
                            nc.tensor.matmul(
                                dps[:], knat[:, col:col + 32],
                                v2n_s[:, 2 * c:2 * c + 2],
                                start=(c == state["delta_done"]),
                                stop=(c == wc[I] - 1))
                        hsb_new = hsbp.tile([32, 2], F32, tag="hsb")
                        # hsb_new = hsb + dps on ACT (one bias-add per
                        # column) — keeps the H chain off the mask-laden
                        # DVE queue
                        for v in range(2):
                            nc.scalar.activation(
                                hsb_new[:, v:v + 1], dps[:, v:v + 1],
                                AF.Identity, bias=hsb[:, v:v + 1])
                        hsb = hsb_new
                        state["delta_done"] = wc[I]
                    hsb_at[I] = hsb

                    # banded S^T = K_band Q_I^T, 4 chunks packed per PSUM
                    # tile
                    nq = (nbl[I] + 3) // 4
                    st4 = [stp.tile([128, 512], F32, tag="st", name=f"st{I}_{q}")
                           for q in range(nq)]
                    for b in range(nbl[I]):
                        c = wc[I] + b
                        nc.tensor.matmul(
                            st4[b // 4][:, 128 * (b % 4):128 * (b % 4) + 128],
                            ktf[:, 128 * c:128 * c + 128],
                            qtf[:, 128 * I:128 * I + 128],
                            start=True, stop=True)
                    st4_at[I] = st4

                def emit_front_masks(I):
                    """Masks for tile I (DVE/ACT/Pool work)."""
                    st4 = st4_at.pop(I)
                    # fused mask+mult: smt = (t1[i] >= t2[j]) * st.
                    # Every mask_pool_every-th op goes to GpSimd via an
                    # Activation-engine SBUF staging copy (GpSimd cannot
                    # read PSUM).
                    smt = smp.tile([128, W], F32, tag="smt")
                    for b in range(nbl[I]):
                        c = wc[I] + b
                        src = st4[b // 4][:, 128 * (b % 4):128 * (b % 4) + 128]
                        use_pool = (mask_pool_every
                                    and state["mask_i"] % mask_pool_every == 0)
                        state["mask_i"] += 1
                        if use_pool:
                            stg = stgp.tile([128, 128], F32, tag="stg")
                            nc.scalar.activation(stg[:], src, AF.Copy)
                            src = stg[:]
                            eng = nc.gpsimd
                        else:
                            eng = nc.vector
                        eng.scalar_tensor_tensor(
                            smt[:, 128 * b:128 * b + 128],
                            t1b_s[:, 128 * I:128 * I + 128],
                            t2p_s[:, c:c + 1],
                            src,
                            OP.is_ge, OP.mult)
                    smt_at[I] = smt

                def emit_back(I):
                    """zot accumulation + writeback for tile I."""
                    smt = smt_at.pop(I)
                    zot = zop.tile([128, 2], F32, tag="zo")
                    nc.tensor.matmul(
                        zot[:], qtf[:, 128 * I:128 * I + 128], hsb_at.pop(I),
                        start=True, stop=False)
                    for b in range(nbl[I]):
                        c = wc[I] + b
                        nc.tensor.matmul(
                            zot[:],
                            smt[:, 128 * b:128 * b + 128],
                            v2n_s[:, 2 * c:2 * c + 2],
                            start=False, stop=(b == nbl[I] - 1))
                    nc.scalar.activation(
                        zsb[:, 2 * I:2 * I + 2], zot[:], AF.Copy)

                # Lazily emit K groups right before the first tile whose
                # band needs their chunks; fronts run `depth` tiles ahead
                # of backs.
                depth = int(os.environ.get("KPIPE", "3"))
                emit_k_l12()
                emitted_P = -1
                pending_T = []
                nxt_front = 0
                nxt_back = 0
                out_done = 0
                while nxt_back < NT:
                    if nxt_front < NT and nxt_front - nxt_back <= depth:
                        # one-group lookahead keeps the PE busy while ACT
                        # writes back kt_s for the transposes
                        need_P = min((wc[nxt_front] + nbl[nxt_front] + 15) // 16, 3)
                        while emitted_P < need_P:
                            emitted_P += 1
                            emit_k_final(emitted_P)
                            pending_T.append(emitted_P)
                        emit_front_st(nxt_front)
                        while pending_T:
                            emit_k_transpose(pending_T.pop(0))
                        emit_front_masks(nxt_front)
                        nxt_front += 1
                    else:
                        emit_back(nxt_back)
                        nxt_back += 1
                        if nxt_back % 8 == 0 and nxt_back < NT:
                            nc.scalar.dma_start(
                                out[:, 2 * out_done:2 * nxt_back],
                                zsb[:, 2 * out_done:2 * nxt_back])
                            out_done = nxt_back

                nc.sync.dma_start(out[:, 2 * out_done:], zsb[:, 2 * out_done:])
    nc.finalize()
    return nc


_CACHE = {}
LAST_RESULTS = None


def kernel(m1, m2, m3, m4, Wq, bq, Wk, bk):
    mods = [np.asarray(m)[0, 0].astype(np.float32) for m in (m1, m2, m3, m4)]
    Wq, bq, Wk, bk = (np.asarray(a, dtype=np.float32) for a in (Wq, bq, Wk, bk))
    t2s = [m[:, -1].copy() for m in mods]
    t1g = mods[0][:, -1].copy()

    # core c: modality c//2, half h=c%2 takes global query tiles 2I+h
    def qsel(h):
        idx = np.arange(TQ)
        gt = 2 * (idx // 128) + h          # global tile
        return gt * 128 + (idx % 128)

    sels = [qsel(0), qsel(1)]
    t1_locals = [t1g[s] for s in sels]
    w, W, nb = _band_meta(t1_locals, t2s)

    key = (tuple(w), W, tuple(nb))
    if key not in _CACHE:
        _CACHE[key] = _build(w, W, nb)
    nc = _CACHE[key]

    def blockdiag(Wl):
        o = np.zeros((128, 128), dtype=np.float32)
        for k in range(4):
            o[32 * k:32 * k + 32, 32 * k:32 * k + 32] = Wl
        return o

    wq_in = np.concatenate([blockdiag(Wq[l]) for l in range(3)], axis=1)
    wk_in = np.concatenate([blockdiag(Wk[l]) for l in range(3)], axis=1)
    bq_in = np.tile(bq.T, (4, 1)).astype(np.float32)
    bk_in = np.tile(bk.T, (4, 1)).astype(np.float32)
    idm_in = np.eye(128, dtype=np.float32)

    in_maps = []
    for c in range(8):
        mod, h = c // 2, c % 2
        x = mods[mod]
        t2 = t2s[mod]
        xk_in = _stack4(np.ascontiguousarray(x.T))
        xq_l = mods[0][sels[h]]
        xq_in = _stack4(np.ascontiguousarray(xq_l.T))
        t1r_in = t1_locals[h].reshape(1, TQ).astype(np.float32)
        t2p_in = np.ascontiguousarray(t2.reshape(NCH, 128).T)
        v2n_in = np.ascontiguousarray(
            x[:, :2].reshape(NCH, 128, 2).transpose(1, 0, 2).reshape(128, 2 * NCH))
        in_maps.append({
            "xk": xk_in, "xq": xq_in, "wk": wk_in, "wq": wq_in,
            "bk": bk_in, "bq": bq_in, "idm": idm_in, "t1r": t1r_in,
            "t2p": t2p_in, "v2n": v2n_in,
        })

    trace = bool(os.environ.get("KERNEL_TRACE"))
    res = run_bass_kernel_spmd(nc, in_maps, core_ids=list(range(8)),
                               trace=trace)
    global LAST_RESULTS
    LAST_RESULTS = res

    y = np.zeros((T, 2), dtype=np.float32)
    for c in range(8):
        mod, h = c // 2, c % 2
        zt = res.results[c]["out"]          # (128, 2*NT): [p, 2I+v]
        zt = zt.reshape(128, NT, 2).transpose(1, 0, 2).reshape(TQ, 2)
        y[sels[h]] += zt
    return y[None, :, :]


# revision 55
# speedup vs baseline: 1.0124x; 1.0124x over previous
"""Trainium2 Bass kernel for sparse_attention problem nn_CAMD_73229192397362.

Reference computation (T=8192, D=32, 4 modalities):
  Q = MLP(m1, Wq, bq); for each modality m: K = MLP(m, Wk, bk), V = m,
  Z_m[i] = sum_{j: t2[j] <= t1[i]} (Q_i . K_j) V_j   (causal linear attention)
  out = (sum_m Z_m)[..., :2]                          -> (1, T, 2)

Only V[:, :2] affects the output, so per modality we need
  Z_m[i, v] = Q_i . H_m(r(i)),  H_m(p) = sum_{j<p} K_j (x) V_j[:2]
with r(i) = #{j : t2[j] <= t1[i]} (both timestamp sequences sorted).

Device algorithm per core (8 cores = 4 modalities x 2 interleaved query
half-sets so band metadata is uniform across cores):
  - MLPs run on the PE in transposed layout (D on partitions) with the
    T-chunks stacked 4-up on the 128 partitions; the three layer weights
    are expanded host-side to 128x128 BLOCK-DIAGONAL matrices so each
    512-column group is ONE matmul instruction (not 4 tile_position ones).
  - K in natural layout (knat) is produced by 16 full 128x128 PE
    transposes of the stacked kt_s (each yields 4 key-chunks at once).
  - Chunked causal attention: for each query tile I (128 queries) the key
    range splits into a full prefix [0, w_I) handled through a running
    (32,2) state H accumulated in PSUM, plus a banded window
    [w_I, w_I + W) where S^T = K_band Q_I^T is computed on PE, masked with
    one fused scalar_tensor_tensor (is_ge + mult) on DVE (the H state
    updates run as per-column bias-adds on ACT to stay off the DVE
    queue), and contracted
    against V2 with output-transposed matmuls zot[128q, 2] (tiny moving
    operand => near-free on PE).  w_I / W are computed host-side from the
    timestamps and are identical for all cores by construction.
  - Single merged instruction stream: a PE warm-up chain covers the HAM
    p-state ramp during the input DMAs; the Q MLP runs first (smaller
    input); the K final layer (+1 group lookahead) and deferred
    transposes are emitted lazily per 512-column group, and attention tiles enter a depth-3 software
    pipeline (S^T/masks run ahead of the zot accumulations) as soon as
    the key chunks their band needs exist.  Input DMAs are spread over
    the SP/ACT/GpSimd queues; t1 is broadcast across partitions on-chip
    by the Pool engine instead of loading a 128x-duplicated tensor.
"""

import os
import numpy as np

import concourse.bass as bass
from concourse.bacc import Bacc
import concourse.mybir as mybir
from concourse.tile import TileContext
from concourse.bass_utils import run_bass_kernel_spmd

T = 8192
D = 32
TQ = 4096          # queries per core
NT = TQ // 128     # query tiles per core (32)
NCH = T // 128     # key chunks (64)
F32 = mybir.dt.float32
AF = mybir.ActivationFunctionType
OP = mybir.AluOpType


def _stack4(xT):
    """(32, Ttot) -> (128, Ttot//4): 512-col chunk g goes to partition
    block g%4, columns (g//4)*512."""
    d, Ttot = xT.shape
    ng = Ttot // 512
    out = np.zeros((128, Ttot // 4), dtype=xT.dtype)
    for g in range(ng):
        k = g % 4
        out[32 * k:32 * k + 32, (g // 4) * 512:(g // 4) * 512 + 512] = \
            xT[:, g * 512:(g + 1) * 512]
    return out


def _knat_col(c):
    """Column offset of key-chunk c inside knat (packed-transpose layout)."""
    u, r = c // 16, c % 16
    J = 4 * u + (r % 4)
    k = r // 4
    return 128 * J + 32 * k


def _band_meta(t1_all, t2_all):
    """Uniform band metadata across all (core, tile) combos.

    t1_all: list over cores of (TQ,) query ts; t2_all: list of (T,) key ts.
    Returns (w, W, nb): w[I] 128-aligned band starts (len NT), band width W,
    nb[I] chunks actually needed per tile."""
    w_raw = np.full(NT, T, dtype=np.int64)
    for t1 in t1_all:
        for t2 in t2_all:
            r_min = np.searchsorted(t2, t1[::128], side="right")  # (NT,)
            w_raw = np.minimum(w_raw, (r_min // 128) * 128)
    W = 0
    for t1 in t1_all:
        for t2 in t2_all:
            r_max = np.searchsorted(t2, t1[127::128], side="right")
            W = max(W, int(np.max(r_max - w_raw)))
    W = max(128, ((W + 127) // 128) * 128)
    w = np.minimum(w_raw, T - W)
    assert np.all(np.diff(w) >= 0)
    for t1 in t1_all:
        for t2 in t2_all:
            r_min = np.searchsorted(t2, t1[::128], side="right")
            r_max = np.searchsorted(t2, t1[127::128], side="right")
            assert np.all(w <= r_min) and np.all(r_max <= w + W)
    nb = np.zeros(NT, dtype=np.int64)
    for t1 in t1_all:
        for t2 in t2_all:
            r_max = np.searchsorted(t2, t1[127::128], side="right")
            nb = np.maximum(nb, (r_max - w + 127) // 128)
    nb = np.maximum(nb, 1)
    return [int(x) for x in w], int(W), [int(x) for x in nb]


def _build(w, W, nb, q0s):
    """Build the Bass module (same program for all 8 cores)."""
    nc = Bacc("TRN2")

    xk = nc.dram_tensor("xk", [128, T // 4], F32, kind="ExternalInput")
    xq = nc.dram_tensor("xq", [128, TQ // 4], F32, kind="ExternalInput")
    wk = nc.dram_tensor("wk", [128, 384], F32, kind="ExternalInput")
    wq = nc.dram_tensor("wq", [128, 384], F32, kind="ExternalInput")
    bk = nc.dram_tensor("bk", [128, 3], F32, kind="ExternalInput")
    bq = nc.dram_tensor("bq", [128, 3], F32, kind="ExternalInput")
    idm = nc.dram_tensor("idm", [128, 128], F32, kind="ExternalInput")
    t1r = nc.dram_tensor("t1r", [1, TQ], F32, kind="ExternalInput")
    t2p = nc.dram_tensor("t2p", [128, NCH], F32, kind="ExternalInput")
    v2n = nc.dram_tensor("v2n", [128, 2 * NCH], F32, kind="ExternalInput")
    # contiguous per-partition layout; host unscrambles to (TQ, 2)
    out = nc.dram_tensor("out", [128, 2 * NT], F32, kind="ExternalOutput")

    NB = W // 128
    wc = [x // 128 for x in w]           # band start chunk per tile
    nbl = nb
    # GPSIMD cannot access PSUM on TRN2, so Pool-offloaded masks first get
    # an Activation-engine staging copy of S^T into SBUF.
    mask_pool_every = int(os.environ.get("KMASK_POOL_EVERY", "0"))

    with TileContext(nc) as tc:
        with tc.tile_pool(name="cst", bufs=1) as cst, \
             tc.tile_pool(name="big", bufs=1) as big, \
             tc.tile_pool(name="hps", bufs=1, space="PSUM") as hps:

            # --- persistent SBUF tensors ---
            wk_s = cst.tile([128, 384], F32)
            wq_s = cst.tile([128, 384], F32)
            bk_s = cst.tile([128, 3], F32)
            bq_s = cst.tile([128, 3], F32)
            idm_s = cst.tile([128, 128], F32)
            t1r_s = cst.tile([1, TQ], F32)
            t1b_s = big.tile([128, TQ], F32, tag="t1b")
            t2p_s = cst.tile([128, NCH], F32)
            v2n_s = cst.tile([128, 2 * NCH], F32)
            xk_s = big.tile([128, T // 4], F32, tag="xk")
            xq_s = big.tile([128, TQ // 4], F32, tag="xq")
            kt_s = big.tile([128, T // 4], F32, tag="kt")    # K^T stacked
            ktf = big.tile([32, T], F32, tag="ktf")          # K^T flat
            knat = big.tile([128, NCH * 32], F32, tag="knat")  # K natural
            qtf = big.tile([32, TQ], F32, tag="qtf")         # Q^T flat
            zsb = cst.tile([128, 2 * NT], F32)

            # Spread input DMAs across the three DMA-capable queues so they
            # issue and transfer concurrently.  Critical path: xq/wq (Q-MLP
            # start) on ACT, xk/wk (K-MLP) on SP, bulk attention inputs via
            # GpSimd's software DGE (slow generation, async transfer).
            for G in range(2):
                nc.scalar.dma_start(xq_s[:, 512 * G:512 * G + 512],
                                    xq[:, 512 * G:512 * G + 512])
            for dst, src in ((wq_s, wq), (bq_s, bq), (wk_s, wk), (bk_s, bk)):
                nc.sync.dma_start(dst[:], src[:])
            # xk in 4 pieces so K-MLP layer 1 starts before the full load
            for G in range(4):
                nc.sync.dma_start(xk_s[:, 512 * G:512 * G + 512],
                                  xk[:, 512 * G:512 * G + 512])
            for dst, src in ((idm_s, idm), (t2p_s, t2p), (v2n_s, v2n),
                             (t1r_s, t1r)):
                nc.gpsimd.dma_start(dst[:], src[:])
            # broadcast t1 across partitions on the (otherwise idle) Pool
            # engine instead of DMAing a 128x-duplicated 2MB tensor
            for G in range(4):
                nc.gpsimd.partition_broadcast(
                    t1b_s[:, 1024 * G:1024 * G + 1024],
                    t1r_s[:, 1024 * G:1024 * G + 1024])

            # ---------------- merged MLP + attention stream ----------------
            # The K-side MLP runs DEPTH-FIRST per 512-column group (each
            # group yields 16 contiguous key chunks of ktf and knat), and
            # attention tiles are admitted into the software pipeline as
            # soon as the key chunks their band needs are ready.  This
            # overlaps the (PE-bound) MLP tail with the (DVE-bound) mask
            # stream of early attention tiles.
            with tc.tile_pool(name="mlp", bufs=2, space="PSUM") as mlp, \
                 tc.tile_pool(name="hbuf", bufs=2) as hbuf, \
                 tc.tile_pool(name="stp", bufs=3, space="PSUM") as stp, \
                 tc.tile_pool(name="zop", bufs=2, space="PSUM") as zop, \
                 tc.tile_pool(name="smp", bufs=6) as smp, \
                 tc.tile_pool(name="stg", bufs=6) as stgp, \
                 tc.tile_pool(name="hsb", bufs=4) as hsbp:

                # PE warm-up chain: data-independent matmuls during the
                # input-DMA wait so the tensor engine's p-state/HAM ramp
                # (~3.4us of throttled clock) is paid before real work.
                nwarm = int(os.environ.get("KWARM", "14"))
                if nwarm:
                    wz = hbuf.tile([128, 64], F32, tag="warm_src", bufs=1)
                    nc.vector.memset(wz[:], 0)
                    wp = mlp.tile([64, 64], F32, tag="mlp")
                    for _ in range(nwarm):
                        nc.tensor.matmul(wp[:], wz[:], wz[:],
                                         start=True, stop=True)

                def hidden_layers(x_s, w_s, b_s, ngrp, use_act):
                    """L1+L2 breadth-first across groups; returns list of
                    per-group h2 tiles.  use_act: relu on the Activation
                    engine (Q side) vs DVE (K side, keeps the ACT queue
                    clear)."""
                    hs = [x_s[:, 512 * G:512 * G + 512] for G in range(ngrp)]
                    for l in range(2):
                        nxt = []
                        for G in range(ngrp):
                            pt = mlp.tile([128, 512], F32, tag="mlp")
                            nc.tensor.matmul(
                                pt[:], w_s[:, 128 * l:128 * l + 128], hs[G],
                                start=True, stop=True)
                            h = hbuf.tile([128, 512], F32, tag="h", bufs=6,
                                          name=f"h{l}_{G}")
                            if use_act:
                                nc.scalar.activation(h[:], pt[:], AF.Relu,
                                                     bias=b_s[:, l:l + 1])
                            else:
                                nc.vector.tensor_scalar(
                                    h[:], pt[:], b_s[:, l:l + 1], 0.0,
                                    OP.add, OP.max)
                            nxt.append(h[:])
                        hs = nxt
                    return hs

                # Q side first: xq is 4x smaller, so the PE starts (and
                # ramps) sooner while xk still streams in.  Stacked group u
                # covers queries [2048u, 2048u+2048); each 32-row piece is
                # contiguous in qtf.
                qts = hbuf.tile([128, TQ // 4], F32, tag="qts")
                h2q = hidden_layers(xq_s, wq_s, bq_s, 2, True)
                for u in range(2):
                    pt = mlp.tile([128, 512], F32, tag="mlp")
                    nc.tensor.matmul(pt[:], wq_s[:, 256:384], h2q[u],
                                     start=True, stop=True)
                    nc.scalar.activation(
                        qts[:, 512 * u:512 * u + 512], pt[:],
                        AF.Identity, bias=bq_s[:, 2:3])
                    for k in range(4):
                        q0 = (4 * u + k) * 512
                        nc.sync.dma_start(
                            qtf[0:32, q0:q0 + 512],
                            qts[32 * k:32 * k + 32, 512 * u:512 * u + 512])

                h2k = [None] * 4

                def emit_k_l12():
                    h2k[:] = hidden_layers(xk_s, wk_s, bk_s, 4, False)

                def emit_k_final(P):
                    """K final layer + unstack for key chunks [16P,16P+16)."""
                    pt = mlp.tile([128, 512], F32, tag="mlp")
                    nc.tensor.matmul(pt[:], wk_s[:, 256:384], h2k[P],
                                     start=True, stop=True)
                    nc.scalar.activation(
                        kt_s[:, 512 * P:512 * P + 512], pt[:],
                        AF.Identity, bias=bk_s[:, 2:3])
                    for k in range(4):
                        k0 = (4 * P + k) * 512
                        nc.sync.dma_start(
                            ktf[0:32, k0:k0 + 512],
                            kt_s[32 * k:32 * k + 32, 512 * P:512 * P + 512])

                def emit_k_transpose(P):
                    """Natural-layout transpose (deferred so other PE work
                    hides the ACT kt_s writeback latency)."""
                    pt = mlp.tile([128, 512], F32, tag="mlp")
                    for j in range(4):
                        J = 4 * P + j
                        nc.tensor.transpose(
                            pt[:, 128 * j:128 * j + 128],
                            kt_s[:, 128 * J:128 * J + 128], idm_s[:])
                    nc.scalar.activation(
                        knat[:, 512 * P:512 * P + 512], pt[:], AF.Copy)

                for i in range(3):
                    stz = stp.tile([128, 512], F32, tag="st", name=f"stz{i}")
                    nc.vector.memset(stz[:], 0)
                hsb = hsbp.tile([32, 2], F32, tag="hsb")
                nc.vector.memset(hsb[:], 0)
                state = {"delta_done": 0, "mask_i": 0}
                hsb_at = {}
                smt_at = {}
                st4_at = {}

                def emit_front_st(I):
                    """H-delta and S^T for tile I (PE work)."""
                    nonlocal hsb
                    if wc[I] > state["delta_done"]:
                        dps = hps.tile([32, 2], F32, tag="dh")
                        for c in range(state["delta_done"], wc[I]):
                            col = _knat_col(c)
                            nc.tensor.matmul(
                                dps[:], knat[:, col:col + 32],
                                v2n_s[:, 2 * c:2 * c + 2],
                                start=(c == state["delta_done"]),
                                stop=(c == wc[I] - 1))
                        hsb_new = hsbp.tile([32, 2], F32, tag="hsb")
                        # hsb_new = hsb + dps on ACT (one bias-add per
                        # column) — keeps the H chain off the mask-laden
                        # DVE queue
                        for v in range(2):
                            nc.scalar.activation(
                                hsb_new[:, v:v + 1], dps[:, v:v + 1],
                                AF.Identity, bias=hsb[:, v:v + 1])
                        hsb = hsb_new
                        state["delta_done"] = wc[I]
                    hsb_at[I] = hsb

                    # banded S^T = K_band Q_I^T, 4 chunks packed per PSUM
                    # tile
                    nq = (nbl[I] + 3) // 4
                    st4 = [stp.tile([128, 512], F32, tag="st", name=f"st{I}_{q}")
                           for q in range(nq)]
                    for b in range(nbl[I]):
                        c = wc[I] + b
                        q0 = q0s[I][b]
                        nc.tensor.matmul(
                            st4[b // 4][:, 128 * (b % 4) + q0:
                                        128 * (b % 4) + 128],
                            ktf[:, 128 * c:128 * c + 128],
                            qtf[:, 128 * I + q0:128 * I + 128],
                            start=True, stop=True)
                    st4_at[I] = st4

                def emit_front_masks(I):
                    """Masks for tile I (DVE/ACT/Pool work)."""
                    st4 = st4_at.pop(I)
                    # fused mask+mult: smt = (t1[i] >= t2[j]) * st.
                    # Every mask_pool_every-th op goes to GpSimd via an
                    # Activation-engine SBUF staging copy (GpSimd cannot
                    # read PSUM).
                    smt = smp.tile([128, W], F32, tag="smt")
                    for b in range(nbl[I]):
                        c = wc[I] + b
                        src = st4[b // 4][:, 128 * (b % 4):128 * (b % 4) + 128]
                        use_pool = (mask_pool_every
                                    and state["mask_i"] % mask_pool_every == 0)
                        state["mask_i"] += 1
                        if use_pool:
                            stg = stgp.tile([128, 128], F32, tag="stg")
                            nc.scalar.activation(stg[:], src, AF.Copy)
                            src = stg[:]
                            eng = nc.gpsimd
                        else:
                            eng = nc.vector
                        eng.scalar_tensor_tensor(
                            smt[:, 128 * b:128 * b + 128],
                            t1b_s[:, 128 * I:128 * I + 128],
                            t2p_s[:, c:c + 1],
                            src,
                            OP.is_ge, OP.mult)
                    smt_at[I] = smt

                def emit_back(I):
                    """zot accumulation + writeback for tile I."""
                    smt = smt_at.pop(I)
                    zot = zop.tile([128, 2], F32, tag="zo")
                    nc.tensor.matmul(
                        zot[:], qtf[:, 128 * I:128 * I + 128], hsb_at.pop(I),
                        start=True, stop=False)
                    for b in range(nbl[I]):
                        c = wc[I] + b
                        nc.tensor.matmul(
                            zot[:],
                            smt[:, 128 * b:128 * b + 128],
                            v2n_s[:, 2 * c:2 * c + 2],
                            start=False, stop=(b == nbl[I] - 1))
                    nc.scalar.activation(
                        zsb[:, 2 * I:2 * I + 2], zot[:], AF.Copy)

                # Lazily emit K groups right before the first tile whose
                # band needs their chunks; fronts run `depth` tiles ahead
                # of backs.
                depth = int(os.environ.get("KPIPE", "3"))
                emit_k_l12()
                emitted_P = -1
                pending_T = []
                nxt_front = 0
                nxt_back = 0
                out_done = 0
                while nxt_back < NT:
                    if nxt_front < NT and nxt_front - nxt_back <= depth:
                        # one-group lookahead keeps the PE busy while ACT
                        # writes back kt_s for the transposes
                        need_P = min((wc[nxt_front] + nbl[nxt_front] + 15) // 16, 3)
                        while emitted_P < need_P:
                            emitted_P += 1
                            emit_k_final(emitted_P)
                            pending_T.append(emitted_P)
                        emit_front_st(nxt_front)
                        while pending_T:
                            emit_k_transpose(pending_T.pop(0))
                        emit_front_masks(nxt_front)
                        nxt_front += 1
                    else:
                        emit_back(nxt_back)
                        nxt_back += 1
                        if nxt_back % 8 == 0 and nxt_back < NT:
                            nc.scalar.dma_start(
                                out[:, 2 * out_done:2 * nxt_back],
                                zsb[:, 2 * out_done:2 * nxt_back])
                            out_done = nxt_back

                nc.sync.dma_start(out[:, 2 * out_done:], zsb[:, 2 * out_done:])
    nc.finalize()
    return nc


_CACHE = {}
LAST_RESULTS = None


def kernel(m1, m2, m3, m4, Wq, bq, Wk, bk):
    mods = [np.asarray(m)[0, 0].astype(np.float32) for m in (m1, m2, m3, m4)]
    Wq, bq, Wk, bk = (np.asarray(a, dtype=np.float32) for a in (Wq, bq, Wk, bk))
    t2s = [m[:, -1].copy() for m in mods]
    t1g = mods[0][:, -1].copy()

    # core c: modality c//2, half h=c%2 takes global query tiles 2I+h
    def qsel(h):
        idx = np.arange(TQ)
        gt = 2 * (idx // 128) + h          # global tile
        return gt * 128 + (idx % 128)

    sels = [qsel(0), qsel(1)]
    t1_locals = [t1g[s] for s in sels]
    w, W, nb = _band_meta(t1_locals, t2s)

    # Per-(tile, chunk) query trim: queries before the first timestamp of
    # a band chunk are fully masked there, so their S^T / mask / zot
    # columns need not be computed.  Must hold for every core -> min.
    q0s = []
    for I in range(NT):
        row = []
        for b in range(nb[I]):
            c = w[I] // 128 + b
            q0 = 128
            for h in range(2):
                t1_tile = t1_locals[h][128 * I:128 * I + 128]
                for t2 in t2s:
                    q0 = min(q0, int(np.searchsorted(t1_tile, t2[128 * c],
                                                     side="left")))
            row.append(min(q0, 127))
        q0s.append(tuple(row))
    q0s = tuple(q0s)

    key = (tuple(w), W, tuple(nb), q0s)
    if key not in _CACHE:
        _CACHE[key] = _build(w, W, nb, q0s)
    nc = _CACHE[key]

    def blockdiag(Wl):
        o = np.zeros((128, 128), dtype=np.float32)
        for k in range(4):
            o[32 * k:32 * k + 32, 32 * k:32 * k + 32] = Wl
        return o

    wq_in = np.concatenate([blockdiag(Wq[l]) for l in range(3)], axis=1)
    wk_in = np.concatenate([blockdiag(Wk[l]) for l in range(3)], axis=1)
    bq_in = np.tile(bq.T, (4, 1)).astype(np.float32)
    bk_in = np.tile(bk.T, (4, 1)).astype(np.float32)
    idm_in = np.eye(128, dtype=np.float32)

    in_maps = []
    for c in range(8):
        mod, h = c // 2, c % 2
        x = mods[mod]
        t2 = t2s[mod]
        xk_in = _stack4(np.ascontiguousarray(x.T))
        xq_l = mods[0][sels[h]]
        xq_in = _stack4(np.ascontiguousarray(xq_l.T))
        t1r_in = t1_locals[h].reshape(1, TQ).astype(np.float32)
        t2p_in = np.ascontiguousarray(t2.reshape(NCH, 128).T)
        v2n_in = np.ascontiguousarray(
            x[:, :2].reshape(NCH, 128, 2).transpose(1, 0, 2).reshape(128, 2 * NCH))
        in_maps.append({
            "xk": xk_in, "xq": xq_in, "wk": wk_in, "wq": wq_in,
            "bk": bk_in, "bq": bq_in, "idm": idm_in, "t1r": t1r_in,
            "t2p": t2p_in, "v2n": v2n_in,
        })

    trace = bool(os.environ.get("KERNEL_TRACE"))
    res = run_bass_kernel_spmd(nc, in_maps, core_ids=list(range(8)),
                               trace=trace)
    global LAST_RESULTS
    LAST_RESULTS = res

    y = np.zeros((T, 2), dtype=np.float32)
    for c in range(8):
        mod, h = c // 2, c % 2
        zt = res.results[c]["out"]          # (128, 2*NT): [p, 2I+v]
        zt = zt.reshape(128, NT, 2).transpose(1, 0, 2).reshape(TQ, 2)
        y[sels[h]] += zt
    return y[None, :, :]
